# revision 16
# baseline (speedup 1.0000x reference)
"""GATEncoder kernel for 8 Trainium2 NeuronCores (bf16 edition).

Strategy (hardcoded for the nn_GATEncoder problem):
  - Only nodes < batch_size (8192) reach the output, so aggregation/decoder
    run for 8192 target nodes, sharded 1024 per core (8 windows of 128).
  - Encoder + GAT projection (xs, a_src, a_dst) are replicated on every core
    for all 10240 (padded) nodes; all matmuls run in bf16 (1 cyc/row on PE
    vs 4 for fp32) with fp32 PSUM accumulation.
  - Each core builds a node-major DRAM table T[10240, 640] bf16 =
    [xs_h0(256) | 1.0 | xs_h1(256) | 1.0 | a_src(2) | pad], in a per-core
    node permutation that puts the core's 1024 target nodes first. The
    constant-1 columns let the softmax denominator z ride along in the
    aggregation matmul (rhs slice is [xs_h | 1]).
  - The GAT projection runs node-major (lhsT = 128-node h2 chunk, rhs =
    gat weights), so no PE transposes are needed to build table rows.
  - Edges are partitioned by target core, bucketed into 8 windows of 128
    targets, sorted by source within a window, padded to a uniform
    per-window tile count (same static schedule on every core).
  - Per 128-edge tile: dma_gather pulls bf16 rows from T; one-hot matrices
    are built on-device from an iota/meta compare; attention softmax and
    scatter-add are one-hot matmuls into PSUM (exact for duplicate edges).
  - Epilogue (skip, ELU, decoder) feature-major on the local 1024 nodes.
"""

import math

import numpy as np
import ml_dtypes

N_NODES = 10000
NPAD = 10240
N_EDGES = 160000
N_IN, H, HEADS, HOUT = 128, 256, 2, 256
BATCH = 8192
NCORES = 8
TPC = BATCH // NCORES          # 1024 target nodes per core
P = 128
WPC = TPC // P                 # 8 windows per core
ROW = 640                      # bf16 table row (1280 B, %256 == 0)
F32 = np.float32
BF16 = ml_dtypes.bfloat16

_cache = {}


# ----------------------------------------------------------------------------
# Host-side preprocessing: edge partitioning / permutation / schedules
# ----------------------------------------------------------------------------

def _prepare_edges(edge_index):
    src = np.asarray(edge_index[0], dtype=np.int64)
    tgt = np.asarray(edge_index[1], dtype=np.int64)
    loops = np.arange(N_NODES, dtype=np.int64)
    src = np.concatenate([src, loops])
    tgt = np.concatenate([tgt, loops])
    keep = tgt < BATCH
    src, tgt = src[keep], tgt[keep]

    core = tgt // TPC
    tloc = tgt - core * TPC
    win = tloc // P
    trel = tloc - win * P

    # per (core, window) edge lists sorted by source
    buckets = {}
    counts = np.zeros((NCORES, WPC), dtype=np.int64)
    for c in range(NCORES):
        m = core == c
        sc, wc, rc = src[m], win[m], trel[m]
        for w in range(WPC):
            mw = wc == w
            s, r = sc[mw], rc[mw]
            o = np.argsort(s, kind="stable")
            buckets[(c, w)] = (s[o], r[o])
            counts[c, w] = s.size

    tiles_per_win = [int(math.ceil(counts[:, w].max() / P)) for w in range(WPC)]
    tiles_per_win = [max(t, 1) for t in tiles_per_win]
    return buckets, tiles_per_win


def _per_core_order(buckets, c):
    """Node permutation: the core's 1024 targets first, then the other
    nodes its edges actually source from (compaction)."""
    targets = np.arange(c * TPC, (c + 1) * TPC, dtype=np.int64)
    need = np.zeros(N_NODES, dtype=bool)
    for w in range(WPC):
        need[buckets[(c, w)][0]] = True
    need[targets] = False
    return np.concatenate([targets, np.nonzero(need)[0]])


def _per_core_arrays(buckets, tiles_per_win, c, order):
    """Returns (gather_idx int16 wrapped, tgt_rel f32 [P, TILES],
    ohT bf16 [TILES, j, p])."""
    ntiles = sum(tiles_per_win)
    srcs = np.zeros(ntiles * P, dtype=np.int64)      # padded slots gather row 0
    trel = np.full(ntiles * P, -1.0, dtype=F32)      # -1 -> contributes nothing
    t0 = 0
    for w in range(WPC):
        s, r = buckets[(c, w)]
        n = s.size
        base = t0 * P
        srcs[base : base + n] = s
        trel[base : base + n] = r.astype(F32)
        t0 += tiles_per_win[w]

    pos = np.zeros(N_NODES, dtype=np.int64)
    pos[order] = np.arange(order.size)

    gidx = pos[srcs].astype(np.int16)                # table row per edge slot
    # wrap int16 indices: element i at [i % 16, i // 16], replicated to 128 rows
    tot = gidx.size
    wrapped = gidx.reshape(tot // 16, 16).T          # [16, tot/16]
    wrapped = np.tile(wrapped, (8, 1)).copy()        # [128, tot/16]

    trel_mat = trel.reshape(ntiles, P).T.copy()              # [P, TILES]

    # transposed one-hot per tile: [t, j, p] = (trel[t, p] == j), bf16
    tr = trel.reshape(ntiles, P)
    iota = np.arange(P, dtype=F32)
    ohT = (tr[:, None, :] == iota[None, :, None]).astype(BF16)  # [T, j, p]
    return wrapped, trel_mat, np.ascontiguousarray(ohT)


# ----------------------------------------------------------------------------
# Bass program
# ----------------------------------------------------------------------------

def _build_program(tiles_per_win, nb):
    import concourse.bacc as bacc
    import concourse.mybir as mybir
    import concourse.tile as tile

    dt = mybir.dt
    Alu = mybir.AluOpType
    Act = mybir.ActivationFunctionType
    BF = dt.bfloat16

    TILES = sum(tiles_per_win)
    NB = nb                     # node blocks in phase A (compacted)
    BN = 512                    # nodes per block
    NT = BN // P                # 4 node chunks of 128 per block
    npad = NB * BN              # compacted node count

    nc = bacc.Bacc("TRN2", target_bir_lowering=False)

    def inp(name, shape, dtype=dt.float32):
        return nc.dram_tensor(name, shape, dtype, kind="ExternalInput")

    xT = inp("xT", [P, npad], BF)
    w1T = inp("w1T", [N_IN, H], BF)
    ln_g = inp("ln_g", [H, 1])
    ln_b = inp("ln_b", [H, 1])
    b1 = inp("b1", [H, 1])
    b2 = inp("b2", [H, 1])
    w2T = inp("w2T", [H, H], BF)
    gatT = inp("gatT", [H, HEADS * HOUT + 4], BF)  # gat_w.T + att cols
    skipT = inp("skipT", [H, HEADS * HOUT], BF)
    gat_bias = inp("gat_bias", [HEADS * HOUT, 1])
    skip_b = inp("skip_b", [HEADS * HOUT, 1])
    d1T = inp("d1T", [HEADS * HOUT, 4 * H], BF)
    db1 = inp("db1", [4 * H, 1])
    d2T = inp("d2T", [4 * H, 1], BF)
    db2 = inp("db2", [1, 1])
    gidx = inp("gidx", [P, (TILES * P) // 16], dt.int16)
    meta = inp("meta", [P, TILES])
    ohT_in = inp("ohT", [TILES, P, P], BF)
    iota_in = inp("iota", [P, P])              # iota[p, j] = j
    negmean_in = inp("negmean", [P, P], BF)    # all -1/256
    posmean_in = inp("posmean", [P, P], BF)    # all +1/256
    ident_in = inp("ident", [P, P], BF)        # identity

    y_out = nc.dram_tensor("y", [1, TPC], dt.float32, kind="ExternalOutput")

    MM = HEADS * HOUT          # 512
    FC = MM // P               # 4 feature chunks of the GAT output
    HW = HOUT + 1              # 257: [xs_h | 1] rhs slice width

    with tile.TileContext(nc) as tc:
        with (
            tc.tile_pool(name="const", bufs=1) as cpool,
            tc.tile_pool(name="persist", bufs=1) as ppool,
            tc.tile_pool(name="dram", bufs=1, space="DRAM") as dpool,
        ):
            # ---- constants / weights to SBUF ----
            def load_const(t, shape, dtype=dt.float32):
                s = cpool.tile(shape, dtype, name=t.name, tag=t.name)
                nc.sync.dma_start(out=s[:], in_=t[:])
                return s

            def load_kc(t, k, cols, dtype=dt.float32):
                """[k*128, cols] DRAM -> [128, k, cols] SBUF."""
                s = cpool.tile([P, k, cols], dtype, name=t.name, tag=t.name)
                nc.sync.dma_start(
                    out=s[:], in_=t[:].rearrange("(k p) c -> p k c", p=P))
                return s

            iota_m = load_const(iota_in, [P, P])
            negmean = load_const(negmean_in, [P, P], BF)
            posmean = load_const(posmean_in, [P, P], BF)
            ident = load_const(ident_in, [P, P], BF)
            w1s = load_const(w1T, [N_IN, H], BF)
            w2s = load_kc(w2T, 2, H, BF)
            gats = load_kc(gatT, 2, MM + 4, BF)
            skips = load_kc(skipT, 2, MM, BF)
            d1s = load_kc(d1T, 4, 4 * H, BF)
            d2s = load_kc(d2T, 8, 1, BF)
            lng = load_kc(ln_g, 2, 1)
            lnb = load_kc(ln_b, 2, 1)
            b1s = load_kc(b1, 2, 1)
            b2s = load_kc(b2, 2, 1)
            gbia = load_kc(gat_bias, 4, 1)
            skb = load_kc(skip_b, 4, 1)
            db1s = load_kc(db1, 8, 1)
            db2s = load_const(db2, [1, 1])
            ln01 = cpool.tile([P, 1], dt.float32, name="ln01", tag="ln01")
            nc.gpsimd.memset(ln01[:], float(np.log(0.1)))
            meta_s = load_const(meta, [P, TILES])
            gidx_s = load_const(gidx, [P, (TILES * P) // 16], dt.int16)

            T_tab = dpool.tile([npad, ROW], BF, name="T_tab",
                               tag="T_tab")

            # persistent: local h2 (skip input), node-major a_dst, agg result
            h2loc = [ppool.tile([P, TPC], BF, name=f"h2loc{m}",
                                tag=f"h2loc{m}") for m in range(2)]
            adstw = ppool.tile([P, 2 * WPC], BF, name="adstw", tag="adstw")
            aggs = ppool.tile([P, WPC, HEADS, HOUT], BF, name="aggs",
                              tag="aggs")

            # ================= Phase A: encoder -> table =================
            with (
                tc.tile_pool(name="wA", bufs=2) as wA,
                tc.tile_pool(name="asmp", bufs=2) as apool,
                tc.tile_pool(name="psA", bufs=2, space="PSUM") as psA,
                tc.tile_pool(name="psA1", bufs=1, space="PSUM") as psA1,
            ):
                for b in range(NB):
                    bsl = slice(b * BN, (b + 1) * BN)
                    xb = wA.tile([P, BN], BF, name="xb", tag="xb")
                    nc.sync.dma_start(out=xb[:], in_=xT[:, bsl])

                    h1 = wA.tile([P, 2, BN], BF, name="h1", tag="h1")
                    sq = wA.tile([P, 2, BN], BF, name="sq", tag="sq")
                    for m in range(2):
                        ps = psA.tile([P, BN], dt.float32, name="psA",
                                      tag="psA")
                        nc.tensor.matmul(
                            ps[:], lhsT=w1s[:, m * P : (m + 1) * P],
                            rhs=xb[:], start=True, stop=True)
                        # h1 evac on Act (bias add), sq on Pool
                        nc.scalar.activation(
                            h1[:, m, :], ps[:], Act.Identity,
                            bias=b1s[:, m, 0:1])
                        nc.vector.scalar_tensor_tensor(
                            sq[:, m, :], h1[:, m, :], 1.0, h1[:, m, :],
                            op0=Alu.mult, op1=Alu.mult)

                    mu = psA1.tile([P, BN], dt.float32, name="muA", tag="muA")
                    ex2 = psA1.tile([P, BN], dt.float32, name="ex2A",
                                    tag="ex2A")
                    for m in range(2):
                        nc.tensor.matmul(mu[:], lhsT=negmean[:],
                                         rhs=h1[:, m, :],
                                         start=(m == 0), stop=(m == 1))
                        nc.tensor.matmul(ex2[:], lhsT=posmean[:],
                                         rhs=sq[:, m, :],
                                         start=(m == 0), stop=(m == 1))
                    # var = (ex2 + eps) - mu^2   (mu holds -mean)
                    musq = wA.tile([P, BN], dt.float32, name="musq",
                                   tag="musq")
                    nc.scalar.activation(musq[:], mu[:], Act.Square)
                    var = wA.tile([P, BN], dt.float32, name="var", tag="var")
                    nc.vector.scalar_tensor_tensor(
                        var[:], ex2[:], 1e-5, musq[:],
                        op0=Alu.add, op1=Alu.subtract)
                    rv = wA.tile([P, BN], dt.float32, name="rv", tag="rv")
                    nc.vector.reciprocal(rv[:], var[:])
                    rstd = wA.tile([P, BN], BF, name="rstd", tag="rstd")
                    nc.scalar.activation(rstd[:], rv[:], Act.Sqrt)

                    hrelu = wA.tile([P, 2, BN], BF, name="hrelu", tag="hrelu")
                    for m in range(2):
                        cen = wA.tile([P, BN], BF, name="cen", tag="cen")
                        nc.vector.tensor_add(cen[:], h1[:, m, :], mu[:])
                        cn = wA.tile([P, BN], BF, name="cn", tag="cn")
                        nc.vector.tensor_mul(cn[:], cen[:], rstd[:])
                        nc.scalar.activation(
                            hrelu[:, m, :], cn[:], Act.Relu,
                            bias=lnb[:, m, 0:1], scale=lng[:, m, 0:1])

                    h2 = wA.tile([P, 2, BN], BF, name="h2", tag="h2")
                    for m in range(2):
                        ps = psA.tile([P, BN], dt.float32, name="psA",
                                      tag="psA")
                        for k in range(2):
                            nc.tensor.matmul(
                                ps[:], lhsT=w2s[:, k, m * P : (m + 1) * P],
                                rhs=hrelu[:, k, :],
                                start=(k == 0), stop=(k == 1))
                        nc.scalar.activation(
                            h2[:, m, :], ps[:], Act.Identity,
                            bias=b2s[:, m, 0:1])

                    if b * BN < TPC:  # blocks covering the local 1024 targets
                        lo = b * BN
                        for m in range(2):
                            nc.vector.tensor_copy(
                                h2loc[m][:, lo : lo + BN], h2[:, m, :])

                    # GAT projection, node-major: one 128-node chunk at a time
                    asm = apool.tile([P, NT, ROW], BF, name="asm", tag="asm")
                    nc.gpsimd.memset(asm[:, :, HOUT : HOUT + 1], 1.0)
                    nc.gpsimd.memset(asm[:, :, MM + 1 : MM + 2], 1.0)
                    for t in range(NT):
                        tsl = slice(t * P, (t + 1) * P)
                        xsps = psA.tile([P, MM], dt.float32, name="xsps",
                                        tag="xsps")
                        for k in range(2):
                            nc.tensor.matmul(
                                xsps[:], lhsT=h2[:, k, tsl],
                                rhs=gats[:, k, 0:MM],
                                start=(k == 0), stop=(k == 1))
                        avps = psA1.tile([P, 4], dt.float32, name="avps",
                                         tag="avps")
                        for k in range(2):
                            nc.tensor.matmul(
                                avps[:], lhsT=h2[:, k, tsl],
                                rhs=gats[:, k, MM : MM + 4],
                                start=(k == 0), stop=(k == 1))
                        # table row: [xs_h0 | 1 | xs_h1 | 1 | a_src | pad]
                        nc.scalar.copy(asm[:, t, 0:HOUT], xsps[:, 0:HOUT])
                        nc.vector.tensor_copy(asm[:, t, HOUT + 1 : MM + 1],
                                              xsps[:, HOUT:MM])
                        nc.vector.tensor_copy(asm[:, t, MM + 2 : MM + 4],
                                              avps[:, 0:2])
                        g = b * NT + t
                        if g < WPC:
                            nc.vector.tensor_copy(
                                adstw[:, 2 * g : 2 * g + 2], avps[:, 2:4])
                    dst = T_tab[:].rearrange("(bb tt pp) r -> bb pp tt r",
                                             bb=NB, pp=P)[b]
                    nc.sync.dma_start(out=dst, in_=asm[:])

            # ================= Phase B: edge aggregation =================
            # Per window: half-window gathers interleaved with their
            # consumers; agg (with fused z column) accumulates in PSUM
            # across the whole window.
            win_t0 = []
            t0 = 0
            for w in range(WPC):
                win_t0.append(t0)
                t0 += tiles_per_win[w]
            GH = 8      # max tiles per gather call (1024 idx = 64 desc/engine)

            def _chunks(base, n):
                k = math.ceil(n / GH)
                sizes = [n // k + (1 if i < n % k else 0) for i in range(k)]
                out, b0 = [], base
                for s in sizes:
                    out.append((b0, s))
                    b0 += s
                return out

            with (
                tc.tile_pool(name="wB", bufs=3) as wB,
                tc.tile_pool(name="gpool", bufs=3) as gpool,
                tc.tile_pool(name="psB", bufs=2, space="PSUM") as psB,
            ):
                osrc = ohT_in[:].rearrange("t j p -> j t p")
                for w in range(WPC):
                    ntw = tiles_per_win[w]
                    halves = _chunks(win_t0[w], ntw)
                    agg = [psB.tile([P, HW], dt.float32, name=f"aggps{h}",
                                    tag=f"aggps{h}") for h in range(HEADS)]
                    done = 0
                    for hb, hn in halves:
                        if hn == 0:
                            continue
                        gb = gpool.tile([P, GH, ROW], BF, name="gb", tag="gb")
                        nc.gpsimd.dma_gather(
                            out_ap=gb[:, :hn, :],
                            in_ap=T_tab[:],
                            idxs_ap=gidx_s[:, hb * 8 : (hb + hn) * 8],
                            num_idxs=hn * P,
                            num_idxs_reg=hn * P,
                            elem_size=ROW,
                        )
                        # one-hot (unweighted): host-built, DMA'd
                        of = wB.tile([P, GH, P], BF, name="of", tag="of")
                        nc.sync.dma_start(out=of[:, :hn, :],
                                          in_=osrc[:, hb : hb + hn, :])

                        dps = psB.tile([P, 2 * GH], dt.float32, name="dps",
                                       tag="dps")
                        for i in range(hn):
                            nc.tensor.matmul(
                                dps[:, 2 * i : 2 * i + 2],
                                lhsT=of[:, i, :],
                                rhs=adstw[:, 2 * w : 2 * w + 2],
                                start=(i == 0), stop=(i == hn - 1),
                                skip_group_check=True)
                        # e = a_src[src] + d ; leaky(0.2); exp
                        esb = wB.tile([P, 2 * GH], dt.float32, name="esb",
                                      tag="esb")
                        nc.vector.tensor_add(
                            esb[:, : 2 * hn].rearrange(
                                "p (t two) -> p t two", two=2),
                            gb[:, :hn, MM + 2 : MM + 4],
                            dps[:, : 2 * hn].rearrange(
                                "p (t two) -> p t two", two=2))
                        lk = wB.tile([P, 2 * GH], dt.float32, name="lk",
                                     tag="lk")
                        nc.vector.scalar_tensor_tensor(
                            lk[:, : 2 * hn], esb[:, : 2 * hn], 0.2,
                            esb[:, : 2 * hn], op0=Alu.mult, op1=Alu.max)
                        wexp = wB.tile([P, 2 * GH], dt.float32, name="wexp",
                                       tag="wexp")
                        nc.scalar.activation(wexp[:, : 2 * hn],
                                             lk[:, : 2 * hn], Act.Exp)

                        # weighted one-hots on DVE (baseline-proven stt form)
                        ohw = wB.tile([P, HEADS, GH, P], BF, name="ohw",
                                      tag="ohw")
                        for i in range(hn):
                            t = hb + i
                            for h in range(HEADS):
                                nc.vector.scalar_tensor_tensor(
                                    ohw[:, h, i, :], iota_m[:],
                                    meta_s[:, t : t + 1],
                                    wexp[:, 2 * i + h : 2 * i + h + 1]
                                    .to_broadcast([P, P]),
                                    op0=Alu.is_equal, op1=Alu.mult)

                        for i in range(hn):
                            for h in range(HEADS):
                                nc.tensor.matmul(
                                    agg[h][:],
                                    lhsT=ohw[:, h, i, :],
                                    rhs=gb[:, i, h * HW : (h + 1) * HW],
                                    start=(done == 0),
                                    stop=(done == ntw - 1),
                                    skip_group_check=True)
                            done += 1
                    # normalize: alpha = w / z  (z rode along in col 256)
                    rz = wB.tile([P, HEADS], dt.float32, name="rz", tag="rz")
                    for h in range(HEADS):
                        nc.vector.reciprocal(rz[:, h : h + 1],
                                             agg[h][:, HOUT : HOUT + 1])
                        nc.vector.tensor_scalar(
                            aggs[:, w, h, :], agg[h][:, 0:HOUT],
                            rz[:, h : h + 1], None, op0=Alu.mult)

            # ================= Phase C: epilogue =================
            with (
                tc.tile_pool(name="wC", bufs=1) as wC,
                tc.tile_pool(name="wC2", bufs=2) as wC2,
                tc.tile_pool(name="psC", bufs=2, space="PSUM") as psC,
                tc.tile_pool(name="psCt", bufs=2, space="PSUM") as psCt,
            ):
                NWC = TPC // 512    # 2 column chunks of 512 nodes
                # aggs node-major [tgt, head, feat] -> convT feature-major
                convT = wC.tile([P, FC, TPC], BF, name="convT", tag="convT")
                for w in range(WPC):
                    for f in range(FC):
                        h, fo = divmod(f * P, HOUT)
                        tp = psCt.tile([P, P], BF, name="tpC", tag="tpC")
                        nc.tensor.transpose(
                            tp[:], aggs[:, w, h, fo : fo + P], ident[:])
                        nc.scalar.activation(
                            convT[:, f, w * P : (w + 1) * P], tp[:],
                            Act.Identity, bias=gbia[:, f, 0:1])

                outT = wC.tile([P, FC, TPC], BF, name="outT", tag="outT")
                for f in range(FC):
                    for n in range(NWC):
                        nsl = slice(n * 512, (n + 1) * 512)
                        sp = psC.tile([P, 512], dt.float32, name="skps",
                                      tag="skps")
                        for k in range(2):
                            nc.tensor.matmul(
                                sp[:], lhsT=skips[:, k, f * P : (f + 1) * P],
                                rhs=h2loc[k][:, nsl],
                                start=(k == 0), stop=(k == 1))
                        t_sb = wC2.tile([P, 512], dt.float32, name="t_sb",
                                        tag="t_sb")
                        nc.vector.scalar_tensor_tensor(
                            t_sb[:], sp[:], skb[:, f, 0:1],
                            convT[:, f, nsl], op0=Alu.add, op1=Alu.add)
                        mn = wC2.tile([P, 512], dt.float32, name="mn",
                                      tag="mn")
                        nc.vector.tensor_scalar_min(mn[:], t_sb[:], 0.0)
                        ez = wC2.tile([P, 512], dt.float32, name="ez",
                                      tag="ez")
                        nc.scalar.activation(ez[:], mn[:], Act.Exp,
                                             bias=ln01[:, 0:1])
                        rl = wC2.tile([P, 512], dt.float32, name="rl",
                                      tag="rl")
                        nc.scalar.activation(rl[:], t_sb[:], Act.Relu)
                        nc.vector.scalar_tensor_tensor(
                            outT[:, f, nsl], ez[:], -0.1, rl[:],
                            op0=Alu.add, op1=Alu.add)

                dsb = wC.tile([P, 8, TPC], BF, name="dsb", tag="dsb")
                for m in range(8):
                    for n in range(NWC):
                        nsl = slice(n * 512, (n + 1) * 512)
                        ps = psC.tile([P, 512], dt.float32, name="decps",
                                      tag="decps")
                        for k in range(FC):
                            nc.tensor.matmul(
                                ps[:], lhsT=d1s[:, k, m * P : (m + 1) * P],
                                rhs=outT[:, k, nsl],
                                start=(k == 0), stop=(k == FC - 1))
                        tmp = wC2.tile([P, 512], dt.float32, name="dtmp",
                                       tag="dtmp")
                        if (m + n) % 2:
                            nc.vector.tensor_scalar(
                                tmp[:], ps[:], db1s[:, m, 0:1], None,
                                op0=Alu.add)
                        else:
                            nc.scalar.activation(
                                tmp[:], ps[:], Act.Identity,
                                bias=db1s[:, m, 0:1])
                        nc.vector.scalar_tensor_tensor(
                            dsb[:, m, nsl], tmp[:], 0.1, tmp[:],
                            op0=Alu.mult, op1=Alu.max)

                ysb = wC.tile([1, TPC], dt.float32, name="ysb", tag="ysb")
                for n in range(NWC):
                    nsl = slice(n * 512, (n + 1) * 512)
                    yp = psC.tile([1, 512], dt.float32, name="yps",
                                  tag="yps")
                    for m in range(8):
                        nc.tensor.matmul(
                            yp[:], lhsT=d2s[:, m, 0:1],
                            rhs=dsb[:, m, nsl],
                            start=(m == 0), stop=(m == 7))
                    nc.scalar.activation(ysb[:, nsl], yp[:], Act.Identity,
                                         bias=db2s[0:1, 0:1])
                nc.sync.dma_start(out=y_out[:], in_=ysb[:])

    nc.compile()
    return nc


# ----------------------------------------------------------------------------
# Driver
# ----------------------------------------------------------------------------

def _consts():
    iota = np.tile(np.arange(P, dtype=F32), (P, 1)).copy()
    negmean = np.full((P, P), -1.0 / H, dtype=BF16)
    posmean = np.full((P, P), 1.0 / H, dtype=BF16)
    ident = np.eye(P, dtype=BF16)
    return iota, negmean, posmean, ident


def _host_in_maps(inputs, buckets, tiles_per_win):
    x = np.asarray(inputs["x"], dtype=F32)
    enc_w1, enc_b1 = inputs["enc_w1"], inputs["enc_b1"]
    ln_g, ln_b = inputs["ln_g"], inputs["ln_b"]
    enc_w2, enc_b2 = inputs["enc_w2"], inputs["enc_b2"]
    gat_w, att_src, att_dst = inputs["gat_w"], inputs["att_src"], inputs["att_dst"]
    gat_bias, skip_w, skip_b = inputs["gat_bias"], inputs["skip_w"], inputs["skip_b"]
    dec_w1, dec_b1 = inputs["dec_w1"], inputs["dec_b1"]
    dec_w2, dec_b2 = inputs["dec_w2"], inputs["dec_b2"]

    # a_src[n,h] = att_src[h] . xs[n,h,:] = (gat_w[h-block].T @ att_src[h]) . h2
    # -> compose the attention vectors into h2-space columns on the host.
    gw = np.asarray(gat_w, F32)
    asr = np.asarray(att_src, F32).reshape(HEADS, HOUT)
    ads = np.asarray(att_dst, F32).reshape(HEADS, HOUT)
    att = np.zeros((H, 4), dtype=F32)
    for h in range(HEADS):
        blk = gw[h * HOUT : (h + 1) * HOUT, :]        # [HOUT, H]
        att[:, h] = blk.T @ asr[h]
        att[:, 2 + h] = blk.T @ ads[h]
    gatT = np.concatenate([np.ascontiguousarray(gw.T), att], axis=1)

    iota, negmean, posmean, ident = _consts()
    col = lambda v: np.ascontiguousarray(np.asarray(v, F32).reshape(-1, 1))
    bf = lambda v: np.ascontiguousarray(np.asarray(v, F32)).astype(BF16)
    common = {
        "w1T": bf(np.asarray(enc_w1, F32).T),
        "ln_g": col(ln_g), "ln_b": col(ln_b),
        "b1": col(enc_b1), "b2": col(enc_b2),
        "w2T": bf(np.asarray(enc_w2, F32).T),
        "gatT": bf(gatT),
        "skipT": bf(np.asarray(skip_w, F32).T),
        "gat_bias": col(gat_bias), "skip_b": col(skip_b),
        "d1T": bf(np.asarray(dec_w1, F32).T),
        "db1": col(dec_b1),
        "d2T": bf(np.asarray(dec_w2, F32).T),
        "db2": col(dec_b2),
        "iota": iota, "negmean": negmean, "posmean": posmean,
        "ident": ident,
    }

    orders = [_per_core_order(buckets, c) for c in range(NCORES)]
    nb = int(math.ceil(max(o.size for o in orders) / 512))
    npad = nb * 512

    in_maps = []
    for c in range(NCORES):
        order = orders[c]
        wrapped, trel_mat, ohT = _per_core_arrays(
            buckets, tiles_per_win, c, order)
        m = dict(common)
        xp = np.zeros((npad, N_IN), dtype=F32)
        xp[:order.size] = x[order]
        m["xT"] = np.ascontiguousarray(xp.T).astype(BF16)
        m["gidx"] = wrapped
        m["meta"] = trel_mat
        m["ohT"] = ohT
        in_maps.append(m)
    return in_maps, nb


def kernel(x, edge_index, batch_size, enc_w1, enc_b1, ln_g, ln_b, enc_w2,
           enc_b2, gat_w, att_src, att_dst, gat_bias, skip_w, skip_b,
           dec_w1, dec_b1, dec_w2, dec_b2, _trace=False):
    edge_index = np.asarray(edge_index)
    buckets, tiles_per_win = _prepare_edges(edge_index)

    inputs = dict(x=x, enc_w1=enc_w1, enc_b1=enc_b1, ln_g=ln_g, ln_b=ln_b,
                  enc_w2=enc_w2, enc_b2=enc_b2, gat_w=gat_w, att_src=att_src,
                  att_dst=att_dst, gat_bias=gat_bias, skip_w=skip_w,
                  skip_b=skip_b, dec_w1=dec_w1, dec_b1=dec_b1, dec_w2=dec_w2,
                  dec_b2=dec_b2)
    in_maps, nb = _host_in_maps(inputs, buckets, tiles_per_win)

    key = (tuple(tiles_per_win), nb)
    if key not in _cache:
        _cache[key] = _build_program(tiles_per_win, nb)
    nc = _cache[key]

    from concourse.bass_utils import run_bass_kernel_spmd
    res = run_bass_kernel_spmd(
        nc, in_maps, core_ids=list(range(NCORES)), trace=_trace)

    y = np.concatenate([res.results[c]["y"][0] for c in range(NCORES)])
    out = y.reshape(BATCH, 1).astype(F32)
    if _trace:
        return out, res
    return out
